# revision 18
# baseline (speedup 1.0000x reference)
"""Directional Chamfer distance kernel for Trainium2 (8 NeuronCores).

Computes sum_m min_n ||t_m - s_n||^2 for template points t (M=10000) and
scan points s (N=20000) in 3D.

Strategy (v4: exact candidate pruning + f32r matmul)
----------------------------------------------------
- Host: KD-sort template points into spatially tight blocks of 128. For
  each block, build a provably sufficient scan candidate set: a scan
  point is kept iff its distance to the (sub-)block bounding box is <=
  an upper bound U on the block's worst nearest-neighbor distance
  (U from seed scan points near the box; exact-NN-preserving by the
  triangle inequality). ~10x fewer distance columns than dense.
- Work is cut into uniform units of 1024 candidate columns (2 chunks of
  512), tagged with their block. Units are distributed round-robin over
  the 8 cores; every core runs the IDENTICAL program (G units), so one
  SPMD program serves all cores. Dummy filler units pad the last cores.
- Device per unit: 2 f32r matmuls (aug K=5 contraction: d2 = t_sq + s_sq
  - 2 t.s as a linear form) -> [128, 1024] PSUM; drain to per-unit
  row-minima. Drain variants: 'scan' (ACT copies half to SBUF, DVE
  tensor_tensor_scan min over psum+sbuf) or 'reduce' (DVE tensor_reduce
  min straight from PSUM).
- Host: gather per-unit minima, min-combine units of the same block,
  clamp at 0 (commutes with min), mask pad rows, sum.
- Fillers use s_sq = 1e30 so padded columns never win the min; f32r
  matmul inputs give ~1e-3 relative accuracy on the final sum (well
  under the 2e-2 gate), at 4x the fp32 matmul throughput.
"""

import os
from contextlib import ExitStack

import numpy as np

import concourse.bacc as bacc
import concourse.tile as tile
from concourse import mybir
from concourse.bass_utils import run_bass_kernel_spmd

N_CORES = 8
NCHUNK = 512            # matmul free dim = one PSUM bank of fp32
KROWS = 32              # contraction rows per PE row-group (5 used)
UNIT = 2 * NCHUNK       # candidate columns per (large) unit
BLOCK = 128             # template rows per block
SUB = 8                 # sub-box granularity for candidate test
NSEED = 64              # seed scan points per sub-box for the U bound
DRAIN = os.environ.get("K_DRAIN", "scan")  # scan | reduce (1024-units)
MM_F32R = os.environ.get("K_F32R", "1") == "1"

FILLER_SSQ = 1.0e30


# --------------------------------------------------------------------------
# host-side planning: KD blocks + exact candidate sets + unit assignment
# --------------------------------------------------------------------------

def _kd_order(pts, leaf):
    out = []

    def rec(ids):
        if len(ids) <= leaf:
            out.append(ids)
            return
        p = pts[ids]
        dim = int(np.argmax(p.max(0) - p.min(0)))
        k = (len(ids) // 2 + leaf - 1) // leaf * leaf
        if k >= len(ids):
            k = len(ids) - leaf
        part = np.argpartition(p[:, dim], k - 1)
        rec(ids[part[:k]])
        rec(ids[part[k:]])

    rec(np.arange(len(pts)))
    return np.concatenate(out)


def _candidates(blk, s):
    """Indices of scan points that can contain the NN of any point in blk.
    Exact: for t in blk with NN s*, d(s*, subbox(t)) <= d(s*, t) = dNN(t)
    <= U(subbox), so s* passes the test. Sub-boxes are PCA-oriented for
    tightness; U comes from seed scan points near the sub-box center."""
    mask = np.zeros(len(s), dtype=bool)
    s64 = s.astype(np.float64)
    for j in range(0, len(blk), SUB):
        sub = blk[j:j + SUB].astype(np.float64)
        c0 = sub.mean(0)
        if len(sub) > 3:
            _, _, V = np.linalg.svd(sub - c0, full_matrices=False)
        else:
            V = np.eye(3)
        subr = (sub - c0) @ V.T
        sr = (s64 - c0) @ V.T
        lo, hi = subr.min(0), subr.max(0)
        c_orig = c0 + ((lo + hi) / 2) @ V
        nseed = min(NSEED, len(s) - 1)
        dc = ((s64 - c_orig) ** 2).sum(1)
        seeds = s64[np.argpartition(dc, nseed)[:nseed]]
        du = np.sqrt(((sub[:, None, :] - seeds[None, :, :]) ** 2).sum(-1))
        U = du.min(1).max() * (1 + 1e-6) + 1e-5
        d_box = np.sqrt(
            (np.maximum(0, np.maximum(lo - sr, sr - hi)) ** 2).sum(1))
        mask |= d_box <= U
    return np.nonzero(mask)[0]


def _plan(scan, template):
    """Returns (order, plan, nblk). plan[c] = (big, small): per-core lists
    of 1024-col and 512-col units, each (block_id, cand_idx | None)."""
    s = np.asarray(scan, dtype=np.float32)
    t = np.asarray(template, dtype=np.float32)
    order = _kd_order(t, BLOCK)
    ts = t[order]
    m = len(t)
    nblk = (m + BLOCK - 1) // BLOCK

    big, small = [], []
    for b in range(nblk):
        blk = ts[b * BLOCK:min((b + 1) * BLOCK, m)]
        cand = _candidates(blk, s)
        off = 0
        while len(cand) - off > NCHUNK:
            big.append((b, cand[off:off + UNIT]))
            off += UNIT
        if len(cand) - off > 0:
            small.append((b, cand[off:]))
    g_big = (len(big) + N_CORES - 1) // N_CORES
    g_small = (len(small) + N_CORES - 1) // N_CORES
    plan = []
    for c in range(N_CORES):
        pb = [big[i] for i in range(c, len(big), N_CORES)]
        psml = [small[i] for i in range(c, len(small), N_CORES)]
        while len(pb) < g_big:
            pb.append((-1, None))
        while len(psml) < g_small:
            psml.append((-1, None))
        plan.append((pb, psml))
    return order, plan, nblk


def _prep_inputs(scan, template, order, plan):
    """Build per-core 'inp' arrays: [2*KROWS, W] combined weights+rhs.
    Layout (columns): [weights big units | weights small units |
    rhs big (chunk j -> row-group j) | rhs small (row-group 0)]."""
    s = np.asarray(scan, dtype=np.float32)
    t = np.asarray(template, dtype=np.float32)
    ts = t[order]
    m = len(t)
    g_big, g_small = len(plan[0][0]), len(plan[0][1])
    wt = (g_big + g_small) * BLOCK
    wr = g_big * NCHUNK + g_small * NCHUNK
    s_sq = (s.astype(np.float64) ** 2).sum(1).astype(np.float32)

    def aug_weights(b):
        aug = np.zeros((KROWS, BLOCK), dtype=np.float32)
        if b >= 0:
            blk = ts[b * BLOCK:min((b + 1) * BLOCK, m)]
            k = len(blk)
            aug[0:3, :k] = -2.0 * blk.T
            aug[3, :k] = 1.0
            aug[4, :k] = (blk.astype(np.float64) ** 2).sum(1)
        return aug

    def rhs_chunk(part):
        rc = np.zeros((KROWS, NCHUNK), dtype=np.float32)
        rc[3, :] = FILLER_SSQ
        rc[4, :] = 1.0
        if part is not None and len(part):
            rc[0:3, :len(part)] = s[part].T
            rc[3, :len(part)] = s_sq[part]
        return rc

    in_maps = []
    for c in range(N_CORES):
        big, small = plan[c]
        inp = np.zeros((2, KROWS, wt + wr), dtype=np.float32)
        for u, (b, cand) in enumerate(big):
            wcol = u * BLOCK
            inp[:, :, wcol:wcol + BLOCK] = aug_weights(b)[None]
            for j in range(2):
                part = cand[j * NCHUNK:(j + 1) * NCHUNK] if cand is not None else None
                col = wt + u * NCHUNK
                inp[j, :, col:col + NCHUNK] = rhs_chunk(part)
        for v, (b, cand) in enumerate(small):
            wcol = (g_big + v) * BLOCK
            inp[:, :, wcol:wcol + BLOCK] = aug_weights(b)[None]
            col = wt + g_big * NCHUNK + v * NCHUNK
            inp[0, :, col:col + NCHUNK] = rhs_chunk(cand)
            # row-group 1 unused for small units; leave filler pattern
            inp[1, 3, col:col + NCHUNK] = FILLER_SSQ
            inp[1, 4, col:col + NCHUNK] = 1.0
        in_maps.append({"inp": inp.reshape(2 * KROWS, wt + wr)})
    return in_maps


# --------------------------------------------------------------------------
# device program
# --------------------------------------------------------------------------

def _build_program(g_big, g_small, repeat=1):
    fp32 = mybir.dt.float32
    mm_dt = mybir.dt.float32r if MM_F32R else fp32
    Alu = mybir.AluOpType
    wt = (g_big + g_small) * BLOCK
    wr = (g_big + g_small) * NCHUNK
    w = wt + wr

    nc = bacc.Bacc("TRN2")
    inp_h = nc.dram_tensor("inp", [2 * KROWS, w], mm_dt, kind="ExternalInput")
    out_h = nc.dram_tensor("out", [128, g_big + g_small], fp32,
                           kind="ExternalOutput")

    with tile.TileContext(nc) as tc:
        with ExitStack() as ctx:
            consts = ctx.enter_context(tc.tile_pool(name="consts", bufs=1))
            # 4 + 4 PSUM banks: two big units + four small units in flight
            pp = ctx.enter_context(
                tc.tile_pool(name="pp", bufs=2, space="PSUM"))
            ps = ctx.enter_context(
                tc.tile_pool(name="ps", bufs=4, space="PSUM"))
            s_pool = ctx.enter_context(tc.tile_pool(name="spool", bufs=4))
            scr_pool = ctx.enter_context(tc.tile_pool(name="scr", bufs=4))

            comb = consts.tile([64, w], mm_dt)
            # split the load so early units can start while the tail streams
            ncut = 4
            for j in range(2):
                for piece in range(ncut):
                    lo = w * piece // ncut
                    hi = w * (piece + 1) // ncut
                    nc.sync.dma_start(
                        out=comb[32 * j:32 * (j + 1), lo:hi],
                        in_=inp_h[KROWS * j:KROWS * (j + 1), lo:hi])

            mins = (consts.tile([128, g_small], fp32, name="mins")
                    if g_small else None)

            def body(_iv=None):
                for u in range(g_big):
                    pt = pp.tile([128, 1024], fp32)
                    for j in range(2):
                        nc.tensor.matmul(
                            out=pt[:, 512 * j:512 * (j + 1)],
                            lhsT=comb[32 * j:32 * (j + 1),
                                      BLOCK * u:BLOCK * (u + 1)],
                            rhs=comb[32 * j:32 * (j + 1),
                                     wt + NCHUNK * u:wt + NCHUNK * (u + 1)],
                            start=True, stop=True,
                            tile_position=(32 * j, 0),
                        )
                    if DRAIN == "reduce":
                        scr = scr_pool.tile([128, 1], fp32)
                        nc.vector.tensor_reduce(
                            out=scr[:, :], in_=pt[:, :],
                            axis=mybir.AxisListType.X, op=Alu.min)
                        nc.sync.dma_start(out=out_h[:, u:u + 1],
                                          in_=scr[:, 0:1])
                    else:
                        st = s_pool.tile([128, 512], fp32)
                        nc.scalar.copy(out=st[:, :], in_=pt[:, 512:1024])
                        scr = scr_pool.tile([128, 512], fp32)
                        nc.vector.tensor_tensor_scan(
                            out=scr[:, :], data0=pt[:, 0:512],
                            data1=st[:, :], initial=3.0e38,
                            op0=Alu.min, op1=Alu.min)
                        # tail column = unit row-minima; DMA it out (free)
                        nc.sync.dma_start(out=out_h[:, u:u + 1],
                                          in_=scr[:, 511:512])
                for v in range(g_small):
                    pt = ps.tile([128, 512], fp32)
                    nc.tensor.matmul(
                        out=pt[:, :],
                        lhsT=comb[0:32, BLOCK * (g_big + v):
                                  BLOCK * (g_big + v + 1)],
                        rhs=comb[0:32, wt + NCHUNK * (g_big + v):
                                 wt + NCHUNK * (g_big + v + 1)],
                        start=True, stop=True,
                        tile_position=(0, 0),
                    )
                    nc.vector.tensor_reduce(
                        out=mins[:, v:v + 1], in_=pt[:, :],
                        axis=mybir.AxisListType.X, op=Alu.min)

            if repeat == 1:
                body()
            else:
                tc.For_i_unrolled(0, repeat, 1, body, max_unroll=1)

            if g_small:
                nc.sync.dma_start(out=out_h[:, g_big:], in_=mins[:, :])
    nc.compile()
    return nc


_CACHE = {}


def _get_program(g_big, g_small, repeat=1):
    key = (g_big, g_small, repeat, DRAIN, MM_F32R)
    if key not in _CACHE:
        _CACHE[key] = _build_program(g_big, g_small, repeat)
    return _CACHE[key]


# --------------------------------------------------------------------------
# public API
# --------------------------------------------------------------------------

def run(scan_vertices, template_vertices, **kw):
    s = np.asarray(scan_vertices, dtype=np.float32)
    t = np.asarray(template_vertices, dtype=np.float32)
    m = len(t)
    order, plan, nblk = _plan(s, t)
    in_maps = _prep_inputs(s, t, order, plan)
    g_big, g_small = len(plan[0][0]), len(plan[0][1])
    nc = _get_program(g_big, g_small)
    res = run_bass_kernel_spmd(nc, in_maps, core_ids=list(range(N_CORES)),
                               **kw)
    # combine: per block, min over its units (and over cores), mask pad rows
    best = np.full((nblk, BLOCK), np.inf, dtype=np.float64)
    for c in range(N_CORES):
        out = res.results[c]["out"]  # [128, g_big + g_small]
        big, small = plan[c]
        for u, (b, cand) in enumerate(big):
            if b >= 0:
                best[b] = np.minimum(best[b], out[:, u].astype(np.float64))
        for v, (b, cand) in enumerate(small):
            if b >= 0:
                best[b] = np.minimum(
                    best[b], out[:, g_big + v].astype(np.float64))
    total = 0.0
    for b in range(nblk):
        k = min(BLOCK, m - b * BLOCK)
        total += np.maximum(best[b, :k], 0.0).sum()
    return np.float32(total), res


def kernel(scan_vertices, template_vertices):
    out, _ = run(scan_vertices, template_vertices)
    return out
